# revision 35
# baseline (speedup 1.0000x reference)
"""GATv2 message passing on 8 Trainium2 NeuronCores (Bass/Tile).

Strategy (edge-parallel by receiver ownership, bf16 compute):
  - Nodes are split into 8 contiguous ranges of 6250; core c owns range c and
    all edges whose receiver falls in it (no cross-core reduction needed).
  - Phase 1: each core projects the full node table with Ws (+Ws_b) and its
    local slice with Wr (+Wr_b) into bf16 DRAM tables.  2048-row groups are
    loaded pre-transposed through the HWDGE xbar (dma_start_transpose), the
    PE runs the projection matmuls, the bias add rides the PSUM->SBUF copy
    on the vector engine, and the table write-back goes out on the scalar
    engine's HWDGE queue so loads and stores issue in parallel.
  - Phase 2: stream the edge shard sorted by (receiver window, sender>=32768),
    with a per-window chunk budget set by the max count across the 8 cores
    (the SPMD program is data-dependent; the compile is cached per graph
    structure).  Sender rows come from a bf16 dma_gather (SWDGE, 4 descriptor
    queues round-robin so desc-gen overlaps DMA drain).  Receiver rows are
    NOT gathered: a window holds only 128 receiver nodes, so each chunk
    selects its rows from the window-resident r-projection tile with a
    one-hot matmul; the sender rows are accumulated into the same PSUM via
    an identity matmul, so x = s_proj + r_proj comes out of PSUM with one
    scalar-engine copy.  mish is exp+square on the scalar engine (both live
    in the exp_and_others activation table, so no table reloads) plus an
    approximate-reciprocal chain on DVE; logits finish with a pairwise-add
    reduce tree (2x DVE mode) and the softmax weight is expanded
    head->features on the scalar engine so the msg multiply also runs in 2x
    mode.  Softmax skips the max pass (logits are O(1) so exp is safe).
    The scatter-add and the softmax denominator share one 136-column matmul
    per chunk into per-window PSUM accumulators, and each window is
    normalized straight out of PSUM and stored.
  - out[n] = segsum(exp(logit)*msg) / segsum(exp(logit)), computed on-device;
    host only reassembles the [50000,128] output from the 8 slices.
"""

import os
import sys

for _p in ("/opt/trn_rl_repo", "/root/.axon_site/_ro/trn_rl_repo"):
    if os.path.isdir(_p) and _p not in sys.path:
        sys.path.insert(0, _p)

import numpy as np
import ml_dtypes

import concourse.bass as bass
import concourse.bacc as bacc
import concourse.tile as tile
from concourse import mybir
from concourse import bass_utils
from concourse.masks import make_identity

F32 = mybir.dt.float32
BF16 = mybir.dt.bfloat16
I16 = mybir.dt.int16
BF = ml_dtypes.bfloat16

N_NODES = 50000
N_EDGES = 800000
F = 128            # feature dim
H = 8              # heads
D = 16             # head dim
NCORE = 8
NPC = N_NODES // NCORE          # 6250 nodes per core
WIN = 128                       # nodes per scatter window
NWIN = (NPC + WIN - 1) // WIN   # 49 windows per core
SPLIT = 32768                   # int16 gather-index limit -> lo/hi tables
NP_PAD = 50176                  # nodes padded to 98*512 for precompute
NL_PAD = 6656                   # local nodes padded to 13*512
HI_ROWS = NP_PAD - SPLIT        # 17408
CHUNK = 128                     # edges per matmul chunk
UNIT = 4                        # chunks per r-select PSUM tile
BLK = 32                        # chunks per DMA block (4096 edges)
GRP = 8                         # max chunks per dma_gather call
NQ = 4                          # SWDGE descriptor queues (round-robin)

_prog_cache = {}


def _chunk_meta(wstruct):
    """Per-chunk (window, table, first, last) from per-window chunk counts.
    Windows alternate lo/hi order so adjacent same-table runs merge into
    longer dma_gather calls."""
    meta = []
    for w, (lw, hw) in enumerate(wstruct):
        cw = lw + hw
        halves = ("lo",) * lw + ("hi",) * hw
        if w % 2 == 1:
            halves = halves[::-1]
        for pos in range(cw):
            meta.append((w, halves[pos], pos == 0, pos == cw - 1))
    return meta


def _build_program(wstruct, nblk, attn_bias):
    """Build the SPMD Bass program for the per-window chunk structure."""
    meta = _chunk_meta(wstruct)
    n_real = len(meta)
    assert nblk * BLK >= n_real

    nc = bacc.Bacc("TRN2", target_bir_lowering=False, debug=False,
                   enable_asserts=False, num_devices=NCORE,
                   num_swdge_queues=NQ)

    def dram_in(name, shape, dt=BF16):
        return nc.dram_tensor(name, list(shape), dt, kind="ExternalInput").ap()

    nodes_padT = dram_in("nodes_padT", (128, NP_PAD))
    nodes_locT = dram_in("nodes_locT", (128, NL_PAD))
    ws_mat = dram_in("ws_mat", (F, F))
    wr_mat = dram_in("wr_mat", (F, F))
    wsb_rep = dram_in("wsb_rep", (128, F))
    wrb_rep = dram_in("wrb_rep", (128, F))
    attn_in = dram_in("attn_rep", (128, 128))
    sidx_in = dram_in("sidx", (nblk, 128, BLK * CHUNK // 16), I16)
    ohh_in = dram_in("ohh", (nblk, 128, BLK * CHUNK))
    ohht_in = dram_in("ohht", (nblk, 128, BLK * CHUNK))
    out_d = nc.dram_tensor("out_d", [NWIN * WIN, F], F32, kind="ExternalOutput").ap()

    tab_lo = nc.dram_tensor("tab_lo", [SPLIT, F], BF16, kind="Internal").ap()
    tab_hi = nc.dram_tensor("tab_hi", [HI_ROWS, F], BF16, kind="Internal").ap()
    tab_r = nc.dram_tensor("tab_r", [NL_PAD, F], BF16, kind="Internal").ap()

    with tile.TileContext(nc) as tc:
        # ---------------- Phase 1: projection precompute ----------------
        with tc.tile_pool(name="pp_const", bufs=1) as cpool, \
             tc.tile_pool(name="pp_sbuf", bufs=3) as spool, \
             tc.tile_pool(name="pp_psum", bufs=2, space="PSUM") as ppool:
            ws_t = cpool.tile([F, F], BF16)
            wr_t = cpool.tile([F, F], BF16)
            wsb_t = cpool.tile([128, F], BF16)
            wrb_t = cpool.tile([128, F], BF16)
            nc.sync.dma_start(out=ws_t[:], in_=ws_mat[:])
            nc.sync.dma_start(out=wr_t[:], in_=wr_mat[:])
            nc.sync.dma_start(out=wsb_t[:], in_=wsb_rep[:])
            nc.sync.dma_start(out=wrb_t[:], in_=wrb_rep[:])

            def project(srcT_ap, src_row, nrows, w_t, b_t, dst_ap, dst_row):
                nch = nrows // 128
                xT = spool.tile([128, nch, 128], BF16, tag=f"pp_x{nrows}")
                nc.sync.dma_start(
                    out=xT[:],
                    in_=srcT_ap[:, src_row:src_row + nrows]
                        .rearrange("p (c k) -> p c k", k=128))
                y = spool.tile([128, nch, 128], BF16, tag=f"pp_y{nrows}")
                for sub in range(nch // 4):
                    pS = ppool.tile([128, 4, 128], F32, space="PSUM",
                                    tag="pp_s")
                    for c in range(4):
                        nc.tensor.matmul(pS[:, c, :],
                                         lhsT=xT[:, sub * 4 + c, :],
                                         rhs=w_t[:], start=True, stop=True,
                                         skip_group_check=True)
                    nc.vector.tensor_tensor(
                        y[:, sub * 4:(sub + 1) * 4, :], pS[:],
                        b_t[:].unsqueeze(1).to_broadcast([128, 4, 128]),
                        op=mybir.AluOpType.add)
                nc.scalar.dma_start(
                    out=dst_ap[dst_row:dst_row + nrows, :]
                        .rearrange("(c p) f -> p c f", p=128),
                    in_=y[:])

            for row in range(0, SPLIT, 2048):
                project(nodes_padT, row, 2048, ws_t, wsb_t, tab_lo, row)
            row = SPLIT
            while row < NP_PAD:
                nrows = min(2048, NP_PAD - row)
                project(nodes_padT, row, nrows, ws_t, wsb_t, tab_hi,
                        row - SPLIT)
                row += nrows
            row = 0
            while row < NL_PAD:
                nrows = min(2048, NL_PAD - row)
                project(nodes_locT, row, nrows, wr_t, wrb_t, tab_r, row)
                row += nrows

        tc.strict_bb_all_engine_barrier()

        # ---------------- Phase 2: edge stream ----------------
        tabs = {"lo": tab_lo, "hi": tab_hi}
        with tc.tile_pool(name="mc", bufs=1) as cpool2, \
             tc.tile_pool(name="gst", bufs=3) as gpool, \
             tc.tile_pool(name="g8", bufs=12) as g8pool, \
             tc.tile_pool(name="stage", bufs=2) as stpool, \
             tc.tile_pool(name="rwinp", bufs=3) as rwpool, \
             tc.tile_pool(name="work", bufs=2) as wpool, \
             tc.tile_pool(name="mid", bufs=1) as mpool, \
             tc.tile_pool(name="psR", bufs=3, space="PSUM") as psR, \
             tc.tile_pool(name="psA", bufs=2, space="PSUM") as psA:
            attn_t = cpool2.tile([128, 128], BF16)
            ident_t = cpool2.tile([128, 128], BF16)
            nc.sync.dma_start(out=attn_t[:], in_=attn_in[:])
            make_identity(nc, ident_t[:])

            acc_ps = None
            rwin_t = None
            qn = [0]

            for b in range(nblk):
                g0 = b * BLK
                rc = min(BLK, n_real - g0)
                if rc <= 0:
                    break
                sidx_t = gpool.tile([128, BLK * CHUNK // 16], I16, tag="sidx")
                oh_t = stpool.tile([128, BLK * CHUNK], BF16, tag="oh")
                ohT_t = stpool.tile([128, BLK * CHUNK], BF16, tag="ohT")
                nc.scalar.dma_start(out=sidx_t[:], in_=sidx_in[b])
                nc.sync.dma_start(out=oh_t[:], in_=ohh_in[b])
                nc.sync.dma_start(out=ohT_t[:], in_=ohht_in[b])
                # gather into per-octet tiles (own buffers) so consecutive
                # dma_gather calls have no WAW on a shared tile and pipeline
                # across the descriptor queues
                noct = (rc + GRP - 1) // GRP
                s8_tiles = []
                for o in range(noct):
                    os0 = o * GRP
                    oc = min(GRP, rc - os0)
                    s8 = g8pool.tile([128, GRP, 128], BF16, tag="s8")
                    s8_tiles.append(s8)
                    cs = 0
                    while cs < oc:
                        t = meta[g0 + os0 + cs][1]
                        ce = cs + 1
                        while ce < oc and meta[g0 + os0 + ce][1] == t:
                            ce += 1
                        nchk = ce - cs
                        nc.gpsimd.dma_gather(
                            out_ap=s8[:, cs:ce, :], in_ap=tabs[t][:],
                            idxs_ap=sidx_t[:, (os0 + cs) * 8:(os0 + ce) * 8],
                            num_idxs=nchk * CHUNK, num_idxs_reg=nchk * CHUNK,
                            elem_size=F, queue_num=qn[0])
                        qn[0] = (qn[0] + 1) % NQ
                        cs = ce

                # per-unit: PSUM x = s_proj (identity mm) + r_proj (one-hot
                # mm against the window tile); x copied out on the scalar
                # engine so DVE ops downstream all run on bf16 SBUF tiles
                x_t = wpool.tile([128, BLK, 128], BF16, tag="x")
                nunit = (rc + UNIT - 1) // UNIT
                for u in range(nunit):
                    c0 = u * UNIT
                    cn = min(UNIT, rc - c0)
                    r_ps = psR.tile([128, UNIT, 128], F32, space="PSUM",
                                    tag="r_ps")
                    s8 = s8_tiles[c0 // GRP]
                    sofs = c0 % GRP
                    nc.tensor.matmul(r_ps[:, :cn, :], lhsT=ident_t[:],
                                     rhs=s8[:, sofs:sofs + cn, :], start=True,
                                     stop=False, skip_group_check=True)
                    for j in range(cn):
                        g = g0 + c0 + j
                        w, _, first, _ = meta[g]
                        if first:
                            rwin_t = rwpool.tile([128, 128], BF16, tag="rwin")
                            nc.sync.dma_start(
                                out=rwin_t[:],
                                in_=tab_r[w * 128:(w + 1) * 128, :])
                        nc.tensor.matmul(
                            r_ps[:, j, :],
                            lhsT=ohT_t[:, (c0 + j) * CHUNK:(c0 + j + 1) * CHUNK],
                            rhs=rwin_t[:], start=False, stop=(j == cn - 1),
                            skip_group_check=True)
                    nc.scalar.activation(x_t[:, c0:c0 + cn, :],
                                         r_ps[:, :cn, :],
                                         mybir.ActivationFunctionType.Copy)

                # mish(x) = x * (1 - 2/((1+e^x)^2+1)): exp+square on the
                # scalar engine (both in the exp_and_others table set), the
                # reciprocal chain on DVE in f32, everything else bf16 2x
                u_t = mpool.tile([128, BLK, 128], BF16, tag="u")
                nc.scalar.activation(u_t[:, :rc, :], x_t[:, :rc, :],
                                     mybir.ActivationFunctionType.Exp)
                q_t = mpool.tile([128, BLK, 128], F32, tag="q")
                nc.scalar.activation(q_t[:, :rc, :], u_t[:, :rc, :],
                                     mybir.ActivationFunctionType.Square,
                                     bias=1.0)
                nc.vector.tensor_scalar_add(q_t[:, :rc, :], q_t[:, :rc, :],
                                            1.0)
                rcp_t = mpool.tile([128, BLK, 128], F32, tag="rcp")
                nc.vector.reciprocal_approx_fast(rcp_t[:, :rc, :],
                                                 q_t[:, :rc, :])
                rr_t = mpool.tile([128, BLK, 128], BF16, tag="rr")
                nc.vector.tensor_scalar(rr_t[:, :rc, :], rcp_t[:, :rc, :],
                                        -2.0, 1.0,
                                        op0=mybir.AluOpType.mult,
                                        op1=mybir.AluOpType.add)
                h_t = mpool.tile([128, BLK, 128], BF16, tag="h")
                nc.vector.tensor_tensor(h_t[:, :rc, :], x_t[:, :rc, :],
                                        rr_t[:, :rc, :],
                                        op=mybir.AluOpType.mult)

                # per-head logits: ha = mish * attn, then pairwise-add tree
                nc.vector.tensor_tensor(
                    h_t[:, :rc, :], h_t[:, :rc, :],
                    attn_t[:].unsqueeze(1).to_broadcast([128, rc, 128]),
                    op=mybir.AluOpType.mult)
                hv = h_t[:, :rc, :].rearrange("p c (h d) -> p c h d", d=D)
                t8 = mpool.tile([128, BLK, H, 8], BF16, tag="t8")
                nc.vector.tensor_tensor(t8[:, :rc], hv[:, :, :, 0:8],
                                        hv[:, :, :, 8:16],
                                        op=mybir.AluOpType.add)
                t4 = mpool.tile([128, BLK, H, 4], BF16, tag="t4")
                nc.vector.tensor_tensor(t4[:, :rc], t8[:, :rc, :, 0:4],
                                        t8[:, :rc, :, 4:8],
                                        op=mybir.AluOpType.add)
                t2 = mpool.tile([128, BLK, H, 2], BF16, tag="t2")
                nc.vector.tensor_tensor(t2[:, :rc], t4[:, :rc, :, 0:2],
                                        t4[:, :rc, :, 2:4],
                                        op=mybir.AluOpType.add)
                lgt_t = mpool.tile([128, BLK, H], BF16, tag="lgt")
                nc.vector.tensor_tensor(lgt_t[:, :rc, :].unsqueeze(3),
                                        t2[:, :rc, :, 0:1], t2[:, :rc, :, 1:2],
                                        op=mybir.AluOpType.add)

                # softmax weight, expanded head->features on the scalar
                # engine so msg runs in 2x DVE mode
                pT128_t = wpool.tile([128, BLK, 128], BF16, tag="pT128")
                nc.scalar.activation(
                    pT128_t[:, :rc, :].rearrange("p c (h d) -> p c h d", d=D),
                    lgt_t[:, :rc, :].unsqueeze(3).to_broadcast([128, rc, H, D]),
                    mybir.ActivationFunctionType.Exp, bias=float(attn_bias))
                rhs_t = wpool.tile([128, BLK, 136], BF16, tag="rhs")
                nc.scalar.activation(rhs_t[:, :rc, 128:136], lgt_t[:, :rc, :],
                                     mybir.ActivationFunctionType.Exp,
                                     bias=float(attn_bias))
                for o in range(noct):
                    os0 = o * GRP
                    oc = min(GRP, rc - os0)
                    nc.vector.tensor_tensor(
                        rhs_t[:, os0:os0 + oc, :128],
                        s8_tiles[o][:, :oc, :],
                        pT128_t[:, os0:os0 + oc, :],
                        op=mybir.AluOpType.mult)

                for cc in range(rc):
                    g = g0 + cc
                    w, _, first, last = meta[g]
                    if first:
                        acc_ps = psA.tile([128, 136], F32, space="PSUM",
                                          tag="agg")
                    nc.tensor.matmul(acc_ps[:],
                                     lhsT=oh_t[:, cc * CHUNK:(cc + 1) * CHUNK],
                                     rhs=rhs_t[:, cc, :], start=first,
                                     stop=last, skip_group_check=True)
                    if last:
                        # normalize straight out of PSUM and store the window
                        denw = mpool.tile([128, H], F32, tag="denw")
                        nc.vector.tensor_scalar_add(denw[:],
                                                    acc_ps[:, 128:136], 1e-30)
                        rcpw = mpool.tile([128, H], F32, tag="rcpw")
                        nc.vector.reciprocal_approx_fast(rcpw[:], denw[:])
                        outw = mpool.tile([128, 128], F32, tag="outw")
                        nc.vector.tensor_tensor(
                            outw[:].rearrange("p (h d) -> p h d", d=D),
                            acc_ps[:, :128].rearrange("p (h d) -> p h d", d=D),
                            rcpw[:].unsqueeze(2).to_broadcast([128, H, D]),
                            op=mybir.AluOpType.mult)
                        nc.sync.dma_start(
                            out=out_d[w * 128:(w + 1) * 128, :], in_=outw[:])

    nc.compile()
    return nc


def _prep_core(senders, receivers, core, wstruct, nblk):
    """Build sidx/ohh/ohht arrays for one core given the chunk structure.
    Edges within each (window, half) run are sorted by sender so the gather
    addresses ascend (HBM row/bank locality)."""
    e_pad = nblk * BLK * CHUNK
    mask = (receivers // NPC) == core
    s = senders[mask].astype(np.int64)
    r = (receivers[mask] - core * NPC).astype(np.int64)
    win = r // WIN
    half = (s >= SPLIT).astype(np.int64)
    order = np.lexsort((s, half, win))
    s, r, win, half = s[order], r[order], win[order], half[order]

    sidx_val = np.zeros(e_pad, np.int64)
    rloc_val = np.full(e_pad, 999.0, np.float32)

    base = 0
    key = win * 2 + half
    for w, (lw, hw) in enumerate(wstruct):
        halves = ((0, lw), (1, hw)) if w % 2 == 0 else ((1, hw), (0, lw))
        for hf, cap_ch in halves:
            gmask = key == (w * 2 + hf)
            n = int(gmask.sum())
            cap = cap_ch * CHUNK
            assert n <= cap, f"window {w} half {hf}: {n} > {cap}"
            if n:
                sg = s[gmask]
                sidx_val[base:base + n] = sg - (SPLIT if hf else 0)
                rloc_val[base:base + n] = (r[gmask] - w * WIN).astype(
                    np.float32)
            base += cap

    def wrap16(vals):
        v = vals.reshape(nblk, BLK * CHUNK // 16, 16).astype(np.int16)
        v = np.transpose(v, (0, 2, 1))          # [nblk, 16, 256]
        return np.tile(v, (1, 8, 1)).copy()     # [nblk, 128, 256]

    sidx = wrap16(sidx_val)
    rv = rloc_val.reshape(nblk, BLK, CHUNK)     # [b, c, p]
    iot = np.arange(128, dtype=np.float32)
    oh = (rv[:, :, :, None] == iot)             # [b, c, p(edge), n]
    ohh = np.ascontiguousarray(
        oh.transpose(0, 2, 1, 3)).reshape(nblk, CHUNK, BLK * 128)
    ohh = ohh.astype(np.float32).astype(BF)
    # transposed one-hot: [b, n(node partition), c*128+edge]
    ohht = np.ascontiguousarray(
        oh.transpose(0, 3, 1, 2)).reshape(nblk, 128, BLK * CHUNK)
    ohht = ohht.astype(np.float32).astype(BF)
    return sidx, ohh, ohht


def kernel(nodes, senders, receivers, Ws_k, Ws_b, Wr_k, Wr_b, attn_k, attn_b):
    nodes = np.asarray(nodes, np.float32)
    senders = np.asarray(senders, np.int32)
    receivers = np.asarray(receivers, np.int32)
    assert nodes.shape == (N_NODES, F) and senders.shape == (N_EDGES,)

    # per-window chunk structure: max count across the 8 cores per window
    core_of = receivers // NPC
    r_loc = receivers - core_of * NPC
    win = r_loc // WIN
    half = (senders >= SPLIT).astype(np.int64)
    key = (core_of.astype(np.int64) * NWIN + win) * 2 + half
    counts = np.bincount(key, minlength=NCORE * NWIN * 2).reshape(
        NCORE, NWIN, 2)
    lo_w = np.ceil(counts[:, :, 0].max(axis=0) / CHUNK).astype(int)
    hi_w = np.ceil(counts[:, :, 1].max(axis=0) / CHUNK).astype(int)
    lo_w = np.maximum(lo_w, 1)          # ensure every window has >=1 chunk
    wstruct = tuple((int(a), int(b)) for a, b in zip(lo_w, hi_w))
    n_real = int(lo_w.sum() + hi_w.sum())
    nblk = (n_real + BLK - 1) // BLK

    ck = (wstruct, nblk, float(np.asarray(attn_b).ravel()[0]))
    if ck not in _prog_cache:
        _prog_cache[ck] = _build_program(*ck)
    nc = _prog_cache[ck]

    nodes_bf = nodes.astype(BF)
    nodes_padT = np.zeros((128, NP_PAD), BF)
    nodes_padT[:, :N_NODES] = nodes_bf.T
    ws_mat = np.asarray(Ws_k, np.float32).reshape(F, F).astype(BF)
    wr_mat = np.asarray(Wr_k, np.float32).reshape(F, F).astype(BF)
    wsb_rep = np.broadcast_to(
        np.asarray(Ws_b, np.float32).reshape(1, F), (128, F)).astype(BF).copy()
    wrb_rep = np.broadcast_to(
        np.asarray(Wr_b, np.float32).reshape(1, F), (128, F)).astype(BF).copy()
    a_flat = np.tile(np.asarray(attn_k, np.float32).ravel(), H)
    attn_rep = np.broadcast_to(a_flat, (128, 128)).astype(BF).copy()

    in_maps = []
    for c in range(NCORE):
        sidx, ohh, ohht = _prep_core(senders, receivers, c, wstruct, nblk)
        nodes_locT = np.zeros((128, NL_PAD), BF)
        nodes_locT[:, :NPC] = nodes_bf[c * NPC:(c + 1) * NPC].T
        in_maps.append({
            "nodes_padT": nodes_padT, "nodes_locT": nodes_locT,
            "ws_mat": ws_mat, "wr_mat": wr_mat,
            "wsb_rep": wsb_rep, "wrb_rep": wrb_rep,
            "attn_rep": attn_rep,
            "sidx": sidx, "ohh": ohh, "ohht": ohht,
        })

    trace = bool(int(os.environ.get("GAT_TRACE", "0")))
    res = bass_utils.run_bass_kernel_spmd(nc, in_maps,
                                          core_ids=list(range(NCORE)),
                                          trace=trace)
    if trace:
        kernel.last_profile = res
    out = np.empty((N_NODES, F), np.float32)
    for c in range(NCORE):
        out[c * NPC:(c + 1) * NPC] = np.asarray(res.results[c]["out_d"])[:NPC]
    return out


# revision 38
# speedup vs baseline: 1.0256x; 1.0256x over previous
"""GATv2 message passing on 8 Trainium2 NeuronCores (Bass/Tile).

Strategy (edge-parallel by receiver ownership, bf16 compute):
  - Nodes are split into 8 contiguous ranges of 6250; core c owns range c and
    all edges whose receiver falls in it (no cross-core reduction needed).
  - Phase 1: each core projects the full node table with Ws (+Ws_b) and its
    local slice with Wr (+Wr_b) into bf16 DRAM tables.  2048-row groups are
    loaded pre-transposed through the HWDGE xbar (dma_start_transpose), the
    PE runs the projection matmuls, the bias add rides the PSUM->SBUF copy
    on the vector engine, and the table write-back goes out on the scalar
    engine's HWDGE queue so loads and stores issue in parallel.
  - Phase 2: stream the edge shard sorted by (receiver window, sender>=32768),
    with a per-window chunk budget set by the max count across the 8 cores
    (the SPMD program is data-dependent; the compile is cached per graph
    structure).  Sender rows come from a bf16 dma_gather (SWDGE, 4 descriptor
    queues round-robin so desc-gen overlaps DMA drain).  Receiver rows are
    NOT gathered: a window holds only 128 receiver nodes, so each chunk
    selects its rows from the window-resident r-projection tile with a
    one-hot matmul; the sender rows are accumulated into the same PSUM via
    an identity matmul, so x = s_proj + r_proj comes out of PSUM with one
    scalar-engine copy.  mish is exp+square on the scalar engine (both live
    in the exp_and_others activation table, so no table reloads) plus an
    approximate-reciprocal chain on DVE; logits finish with a pairwise-add
    reduce tree (2x DVE mode) and the softmax weight is expanded
    head->features on the scalar engine so the msg multiply also runs in 2x
    mode.  Softmax skips the max pass (logits are O(1) so exp is safe).
    The scatter-add and the softmax denominator share one 136-column matmul
    per chunk into per-window PSUM accumulators, and each window is
    normalized straight out of PSUM and stored.
  - out[n] = segsum(exp(logit)*msg) / segsum(exp(logit)), computed on-device;
    host only reassembles the [50000,128] output from the 8 slices.
"""

import os
import sys

for _p in ("/opt/trn_rl_repo", "/root/.axon_site/_ro/trn_rl_repo"):
    if os.path.isdir(_p) and _p not in sys.path:
        sys.path.insert(0, _p)

import numpy as np
import ml_dtypes

import concourse.bass as bass
import concourse.bacc as bacc
import concourse.tile as tile
from concourse import mybir
from concourse import bass_utils
from concourse.masks import make_identity

F32 = mybir.dt.float32
BF16 = mybir.dt.bfloat16
I16 = mybir.dt.int16
BF = ml_dtypes.bfloat16

N_NODES = 50000
N_EDGES = 800000
F = 128            # feature dim
H = 8              # heads
D = 16             # head dim
NCORE = 8
NPC = N_NODES // NCORE          # 6250 nodes per core
WIN = 128                       # nodes per scatter window
NWIN = (NPC + WIN - 1) // WIN   # 49 windows per core
SPLIT = 32768                   # int16 gather-index limit -> lo/hi tables
NP_PAD = 50176                  # nodes padded to 98*512 for precompute
NL_PAD = 6656                   # local nodes padded to 13*512
HI_ROWS = NP_PAD - SPLIT        # 17408
CHUNK = 128                     # edges per matmul chunk
UNIT = 4                        # chunks per r-select PSUM tile
BLK = 32                        # chunks per DMA block (4096 edges)
GRP = 8                         # max chunks per dma_gather call
NQ = 4                          # SWDGE descriptor queues (round-robin)

_prog_cache = {}


def _chunk_meta(wstruct):
    """Per-chunk (window, table, first, last) from per-window chunk counts.
    Windows alternate lo/hi order so adjacent same-table runs merge into
    longer dma_gather calls."""
    meta = []
    for w, (lw, hw) in enumerate(wstruct):
        cw = lw + hw
        halves = ("lo",) * lw + ("hi",) * hw
        if w % 2 == 1:
            halves = halves[::-1]
        for pos in range(cw):
            meta.append((w, halves[pos], pos == 0, pos == cw - 1))
    return meta


def _build_program(wstruct, nblk, attn_bias):
    """Build the SPMD Bass program for the per-window chunk structure."""
    meta = _chunk_meta(wstruct)
    n_real = len(meta)
    assert nblk * BLK >= n_real

    nc = bacc.Bacc("TRN2", target_bir_lowering=False, debug=False,
                   enable_asserts=False, num_devices=NCORE,
                   num_swdge_queues=NQ)

    def dram_in(name, shape, dt=BF16):
        return nc.dram_tensor(name, list(shape), dt, kind="ExternalInput").ap()

    nodes_padT = dram_in("nodes_padT", (128, NP_PAD))
    nodes_locT = dram_in("nodes_locT", (128, NL_PAD))
    ws_mat = dram_in("ws_mat", (F, F))
    wr_mat = dram_in("wr_mat", (F, F))
    wsb_rep = dram_in("wsb_rep", (128, F))
    wrb_rep = dram_in("wrb_rep", (128, F))
    attn_in = dram_in("attn_rep", (128, 128))
    sidx_in = dram_in("sidx", (nblk, 128, BLK * CHUNK // 16), I16)
    ohh_in = dram_in("ohh", (nblk, 128, BLK * CHUNK))
    ohht_in = dram_in("ohht", (nblk, 128, BLK * CHUNK))
    out_d = nc.dram_tensor("out_d", [NWIN * WIN, F], F32, kind="ExternalOutput").ap()

    tab_lo = nc.dram_tensor("tab_lo", [SPLIT, F], BF16, kind="Internal").ap()
    tab_hi = nc.dram_tensor("tab_hi", [HI_ROWS, F], BF16, kind="Internal").ap()
    tab_r = nc.dram_tensor("tab_r", [NL_PAD, F], BF16, kind="Internal").ap()

    with tile.TileContext(nc) as tc:
        # ---------------- Phase 1: projection precompute ----------------
        with tc.tile_pool(name="pp_const", bufs=1) as cpool, \
             tc.tile_pool(name="pp_sbuf", bufs=3) as spool, \
             tc.tile_pool(name="pp_psum", bufs=2, space="PSUM") as ppool:
            ws_t = cpool.tile([F, F], BF16)
            wr_t = cpool.tile([F, F], BF16)
            wsb_t = cpool.tile([128, F], BF16)
            wrb_t = cpool.tile([128, F], BF16)
            nc.sync.dma_start(out=ws_t[:], in_=ws_mat[:])
            nc.sync.dma_start(out=wr_t[:], in_=wr_mat[:])
            nc.sync.dma_start(out=wsb_t[:], in_=wsb_rep[:])
            nc.sync.dma_start(out=wrb_t[:], in_=wrb_rep[:])

            def project(srcT_ap, src_row, nrows, w_t, b_t, dst_ap, dst_row):
                nch = nrows // 128
                xT = spool.tile([128, nch, 128], BF16, tag=f"pp_x{nrows}")
                nc.sync.dma_start(
                    out=xT[:],
                    in_=srcT_ap[:, src_row:src_row + nrows]
                        .rearrange("p (c k) -> p c k", k=128))
                y = spool.tile([128, nch, 128], BF16, tag=f"pp_y{nrows}")
                for sub in range(nch // 4):
                    pS = ppool.tile([128, 4, 128], F32, space="PSUM",
                                    tag="pp_s")
                    for c in range(4):
                        nc.tensor.matmul(pS[:, c, :],
                                         lhsT=xT[:, sub * 4 + c, :],
                                         rhs=w_t[:], start=True, stop=True,
                                         skip_group_check=True)
                    nc.vector.tensor_tensor(
                        y[:, sub * 4:(sub + 1) * 4, :], pS[:],
                        b_t[:].unsqueeze(1).to_broadcast([128, 4, 128]),
                        op=mybir.AluOpType.add)
                nc.scalar.dma_start(
                    out=dst_ap[dst_row:dst_row + nrows, :]
                        .rearrange("(c p) f -> p c f", p=128),
                    in_=y[:])

            for row in range(0, SPLIT, 2048):
                project(nodes_padT, row, 2048, ws_t, wsb_t, tab_lo, row)
            row = SPLIT
            while row < NP_PAD:
                nrows = min(2048, NP_PAD - row)
                project(nodes_padT, row, nrows, ws_t, wsb_t, tab_hi,
                        row - SPLIT)
                row += nrows
            row = 0
            while row < NL_PAD:
                nrows = min(2048, NL_PAD - row)
                project(nodes_locT, row, nrows, wr_t, wrb_t, tab_r, row)
                row += nrows

        tc.strict_bb_all_engine_barrier()

        # ---------------- Phase 2: edge stream ----------------
        tabs = {"lo": tab_lo, "hi": tab_hi}
        with tc.tile_pool(name="mc", bufs=1) as cpool2, \
             tc.tile_pool(name="gst", bufs=3) as gpool, \
             tc.tile_pool(name="g8", bufs=12) as g8pool, \
             tc.tile_pool(name="stage", bufs=2) as stpool, \
             tc.tile_pool(name="rwinp", bufs=3) as rwpool, \
             tc.tile_pool(name="work", bufs=2) as wpool, \
             tc.tile_pool(name="mid", bufs=1) as mpool, \
             tc.tile_pool(name="psR", bufs=3, space="PSUM") as psR, \
             tc.tile_pool(name="psA", bufs=2, space="PSUM") as psA:
            attn_t = cpool2.tile([128, 128], BF16)
            ident_t = cpool2.tile([128, 128], BF16)
            nc.sync.dma_start(out=attn_t[:], in_=attn_in[:])
            make_identity(nc, ident_t[:])

            acc_ps = None
            rwin_t = None
            qn = [0]

            for b in range(nblk):
                g0 = b * BLK
                rc = min(BLK, n_real - g0)
                if rc <= 0:
                    break
                sidx_t = gpool.tile([128, BLK * CHUNK // 16], I16, tag="sidx")
                oh_t = stpool.tile([128, BLK * CHUNK], BF16, tag="oh")
                ohT_t = stpool.tile([128, BLK * CHUNK], BF16, tag="ohT")
                nc.scalar.dma_start(out=sidx_t[:], in_=sidx_in[b])
                nc.sync.dma_start(out=oh_t[:], in_=ohh_in[b])
                nc.sync.dma_start(out=ohT_t[:], in_=ohht_in[b])
                # gather into per-octet tiles (own buffers) so consecutive
                # dma_gather calls have no WAW on a shared tile and pipeline
                # across the descriptor queues
                noct = (rc + GRP - 1) // GRP
                s8_tiles = []
                for o in range(noct):
                    os0 = o * GRP
                    oc = min(GRP, rc - os0)
                    s8 = g8pool.tile([128, GRP, 128], BF16, tag="s8")
                    s8_tiles.append(s8)
                    cs = 0
                    while cs < oc:
                        t = meta[g0 + os0 + cs][1]
                        ce = cs + 1
                        while ce < oc and meta[g0 + os0 + ce][1] == t:
                            ce += 1
                        nchk = ce - cs
                        nc.gpsimd.dma_gather(
                            out_ap=s8[:, cs:ce, :], in_ap=tabs[t][:],
                            idxs_ap=sidx_t[:, (os0 + cs) * 8:(os0 + ce) * 8],
                            num_idxs=nchk * CHUNK, num_idxs_reg=nchk * CHUNK,
                            elem_size=F, queue_num=qn[0])
                        qn[0] = (qn[0] + 1) % NQ
                        cs = ce

                # per-unit: PSUM x = s_proj (identity mm) + r_proj (one-hot
                # mm against the window tile); x copied out on the scalar
                # engine so DVE ops downstream all run on bf16 SBUF tiles
                x_t = wpool.tile([128, BLK, 128], BF16, tag="x")
                nunit = (rc + UNIT - 1) // UNIT
                for u in range(nunit):
                    c0 = u * UNIT
                    cn = min(UNIT, rc - c0)
                    r_ps = psR.tile([128, UNIT, 128], F32, space="PSUM",
                                    tag="r_ps")
                    s8 = s8_tiles[c0 // GRP]
                    sofs = c0 % GRP
                    nc.tensor.matmul(r_ps[:, :cn, :], lhsT=ident_t[:],
                                     rhs=s8[:, sofs:sofs + cn, :], start=True,
                                     stop=False, skip_group_check=True)
                    for j in range(cn):
                        g = g0 + c0 + j
                        w, _, first, _ = meta[g]
                        if first:
                            rwin_t = rwpool.tile([128, 128], BF16, tag="rwin")
                            nc.sync.dma_start(
                                out=rwin_t[:],
                                in_=tab_r[w * 128:(w + 1) * 128, :])
                        nc.tensor.matmul(
                            r_ps[:, j, :],
                            lhsT=ohT_t[:, (c0 + j) * CHUNK:(c0 + j + 1) * CHUNK],
                            rhs=rwin_t[:], start=False, stop=(j == cn - 1),
                            skip_group_check=True)
                    # x out of PSUM; alternate engines to balance load.
                    # NOTE: no SBUF-source tensor_scalar/tensor_copy on DVE
                    # anywhere in phase 2 -- 2-port DVE perf modes lock GpSimd
                    # out of the shared SBUF port and starve SWDGE desc-gen.
                    if u % 2 == 0:
                        nc.scalar.activation(x_t[:, c0:c0 + cn, :],
                                             r_ps[:, :cn, :],
                                             mybir.ActivationFunctionType.Copy)
                    else:
                        nc.vector.tensor_copy(x_t[:, c0:c0 + cn, :],
                                              r_ps[:, :cn, :])

                # mish(x) = x * (1 - 2/((1+e^x)^2+1)): exp/square/+1/affine on
                # the scalar engine (all in the exp_and_others table set), the
                # approximate reciprocal on DVE (1x custom op), rest bf16 TT
                u_t = mpool.tile([128, BLK, 128], BF16, tag="u")
                nc.scalar.activation(u_t[:, :rc, :], x_t[:, :rc, :],
                                     mybir.ActivationFunctionType.Exp)
                q_t = mpool.tile([128, BLK, 128], F32, tag="q")
                nc.scalar.activation(q_t[:, :rc, :], u_t[:, :rc, :],
                                     mybir.ActivationFunctionType.Square,
                                     bias=1.0)
                nc.scalar.activation(q_t[:, :rc, :], q_t[:, :rc, :],
                                     mybir.ActivationFunctionType.Copy,
                                     bias=1.0)
                rcp_t = mpool.tile([128, BLK, 128], F32, tag="rcp")
                nc.vector.reciprocal_approx_fast(rcp_t[:, :rc, :],
                                                 q_t[:, :rc, :])
                rr_t = mpool.tile([128, BLK, 128], BF16, tag="rr")
                nc.scalar.activation(rr_t[:, :rc, :], rcp_t[:, :rc, :],
                                     mybir.ActivationFunctionType.Copy,
                                     bias=1.0, scale=-2.0)
                h_t = mpool.tile([128, BLK, 128], BF16, tag="h")
                nc.vector.tensor_tensor(h_t[:, :rc, :], x_t[:, :rc, :],
                                        rr_t[:, :rc, :],
                                        op=mybir.AluOpType.mult)

                # per-head logits: ha = mish * attn, then pairwise-add tree
                nc.vector.tensor_tensor(
                    h_t[:, :rc, :], h_t[:, :rc, :],
                    attn_t[:].unsqueeze(1).to_broadcast([128, rc, 128]),
                    op=mybir.AluOpType.mult)
                hv = h_t[:, :rc, :].rearrange("p c (h d) -> p c h d", d=D)
                t8 = mpool.tile([128, BLK, H, 8], BF16, tag="t8")
                nc.vector.tensor_tensor(t8[:, :rc], hv[:, :, :, 0:8],
                                        hv[:, :, :, 8:16],
                                        op=mybir.AluOpType.add)
                t4 = mpool.tile([128, BLK, H, 4], BF16, tag="t4")
                nc.vector.tensor_tensor(t4[:, :rc], t8[:, :rc, :, 0:4],
                                        t8[:, :rc, :, 4:8],
                                        op=mybir.AluOpType.add)
                t2 = mpool.tile([128, BLK, H, 2], BF16, tag="t2")
                nc.vector.tensor_tensor(t2[:, :rc], t4[:, :rc, :, 0:2],
                                        t4[:, :rc, :, 2:4],
                                        op=mybir.AluOpType.add)
                lgt_t = mpool.tile([128, BLK, H], BF16, tag="lgt")
                nc.vector.tensor_tensor(lgt_t[:, :rc, :].unsqueeze(3),
                                        t2[:, :rc, :, 0:1], t2[:, :rc, :, 1:2],
                                        op=mybir.AluOpType.add)

                # softmax weight + combined scatter rhs; msg multiplies the
                # gathered rows by the per-head weight (broadcast along d)
                rhs_t = wpool.tile([128, BLK, 136], BF16, tag="rhs")
                nc.scalar.activation(rhs_t[:, :rc, 128:136], lgt_t[:, :rc, :],
                                     mybir.ActivationFunctionType.Exp,
                                     bias=float(attn_bias))
                for o in range(noct):
                    os0 = o * GRP
                    oc = min(GRP, rc - os0)
                    nc.vector.tensor_tensor(
                        rhs_t[:, os0:os0 + oc, :128]
                            .rearrange("p c (h d) -> p c h d", d=D),
                        s8_tiles[o][:, :oc, :]
                            .rearrange("p c (h d) -> p c h d", d=D),
                        rhs_t[:, os0:os0 + oc, 128:136].unsqueeze(3)
                            .to_broadcast([128, oc, H, D]),
                        op=mybir.AluOpType.mult)

                for cc in range(rc):
                    g = g0 + cc
                    w, _, first, last = meta[g]
                    if first:
                        acc_ps = psA.tile([128, 136], F32, space="PSUM",
                                          tag="agg")
                    nc.tensor.matmul(acc_ps[:],
                                     lhsT=oh_t[:, cc * CHUNK:(cc + 1) * CHUNK],
                                     rhs=rhs_t[:, cc, :], start=first,
                                     stop=last, skip_group_check=True)
                    if last:
                        # normalize straight out of PSUM and store the window
                        denw = mpool.tile([128, H], F32, tag="denw")
                        nc.vector.tensor_scalar_add(denw[:],
                                                    acc_ps[:, 128:136], 1e-30)
                        rcpw = mpool.tile([128, H], F32, tag="rcpw")
                        nc.vector.reciprocal_approx_fast(rcpw[:], denw[:])
                        outw = mpool.tile([128, 128], F32, tag="outw")
                        nc.vector.tensor_tensor(
                            outw[:].rearrange("p (h d) -> p h d", d=D),
                            acc_ps[:, :128].rearrange("p (h d) -> p h d", d=D),
                            rcpw[:].unsqueeze(2).to_broadcast([128, H, D]),
                            op=mybir.AluOpType.mult)
                        nc.sync.dma_start(
                            out=out_d[w * 128:(w + 1) * 128, :], in_=outw[:])

    nc.compile()
    return nc


def _prep_core(senders, receivers, core, wstruct, nblk):
    """Build sidx/ohh/ohht arrays for one core given the chunk structure.
    Edges within each (window, half) run are sorted by sender so the gather
    addresses ascend (HBM row/bank locality)."""
    e_pad = nblk * BLK * CHUNK
    mask = (receivers // NPC) == core
    s = senders[mask].astype(np.int64)
    r = (receivers[mask] - core * NPC).astype(np.int64)
    win = r // WIN
    half = (s >= SPLIT).astype(np.int64)
    order = np.lexsort((s, half, win))
    s, r, win, half = s[order], r[order], win[order], half[order]

    sidx_val = np.zeros(e_pad, np.int64)
    rloc_val = np.full(e_pad, 999.0, np.float32)

    base = 0
    key = win * 2 + half
    for w, (lw, hw) in enumerate(wstruct):
        halves = ((0, lw), (1, hw)) if w % 2 == 0 else ((1, hw), (0, lw))
        for hf, cap_ch in halves:
            gmask = key == (w * 2 + hf)
            n = int(gmask.sum())
            cap = cap_ch * CHUNK
            assert n <= cap, f"window {w} half {hf}: {n} > {cap}"
            if n:
                sg = s[gmask]
                sidx_val[base:base + n] = sg - (SPLIT if hf else 0)
                rloc_val[base:base + n] = (r[gmask] - w * WIN).astype(
                    np.float32)
            base += cap

    def wrap16(vals):
        v = vals.reshape(nblk, BLK * CHUNK // 16, 16).astype(np.int16)
        v = np.transpose(v, (0, 2, 1))          # [nblk, 16, 256]
        return np.tile(v, (1, 8, 1)).copy()     # [nblk, 128, 256]

    sidx = wrap16(sidx_val)
    rv = rloc_val.reshape(nblk, BLK, CHUNK)     # [b, c, p]
    iot = np.arange(128, dtype=np.float32)
    oh = (rv[:, :, :, None] == iot)             # [b, c, p(edge), n]
    ohh = np.ascontiguousarray(
        oh.transpose(0, 2, 1, 3)).reshape(nblk, CHUNK, BLK * 128)
    ohh = ohh.astype(np.float32).astype(BF)
    # transposed one-hot: [b, n(node partition), c*128+edge]
    ohht = np.ascontiguousarray(
        oh.transpose(0, 3, 1, 2)).reshape(nblk, 128, BLK * CHUNK)
    ohht = ohht.astype(np.float32).astype(BF)
    return sidx, ohh, ohht


def kernel(nodes, senders, receivers, Ws_k, Ws_b, Wr_k, Wr_b, attn_k, attn_b):
    nodes = np.asarray(nodes, np.float32)
    senders = np.asarray(senders, np.int32)
    receivers = np.asarray(receivers, np.int32)
    assert nodes.shape == (N_NODES, F) and senders.shape == (N_EDGES,)

    # per-window chunk structure: max count across the 8 cores per window
    core_of = receivers // NPC
    r_loc = receivers - core_of * NPC
    win = r_loc // WIN
    half = (senders >= SPLIT).astype(np.int64)
    key = (core_of.astype(np.int64) * NWIN + win) * 2 + half
    counts = np.bincount(key, minlength=NCORE * NWIN * 2).reshape(
        NCORE, NWIN, 2)
    lo_w = np.ceil(counts[:, :, 0].max(axis=0) / CHUNK).astype(int)
    hi_w = np.ceil(counts[:, :, 1].max(axis=0) / CHUNK).astype(int)
    lo_w = np.maximum(lo_w, 1)          # ensure every window has >=1 chunk
    wstruct = tuple((int(a), int(b)) for a, b in zip(lo_w, hi_w))
    n_real = int(lo_w.sum() + hi_w.sum())
    nblk = (n_real + BLK - 1) // BLK

    ck = (wstruct, nblk, float(np.asarray(attn_b).ravel()[0]))
    if ck not in _prog_cache:
        _prog_cache[ck] = _build_program(*ck)
    nc = _prog_cache[ck]

    nodes_bf = nodes.astype(BF)
    nodes_padT = np.zeros((128, NP_PAD), BF)
    nodes_padT[:, :N_NODES] = nodes_bf.T
    ws_mat = np.asarray(Ws_k, np.float32).reshape(F, F).astype(BF)
    wr_mat = np.asarray(Wr_k, np.float32).reshape(F, F).astype(BF)
    wsb_rep = np.broadcast_to(
        np.asarray(Ws_b, np.float32).reshape(1, F), (128, F)).astype(BF).copy()
    wrb_rep = np.broadcast_to(
        np.asarray(Wr_b, np.float32).reshape(1, F), (128, F)).astype(BF).copy()
    a_flat = np.tile(np.asarray(attn_k, np.float32).ravel(), H)
    attn_rep = np.broadcast_to(a_flat, (128, 128)).astype(BF).copy()

    in_maps = []
    for c in range(NCORE):
        sidx, ohh, ohht = _prep_core(senders, receivers, c, wstruct, nblk)
        nodes_locT = np.zeros((128, NL_PAD), BF)
        nodes_locT[:, :NPC] = nodes_bf[c * NPC:(c + 1) * NPC].T
        in_maps.append({
            "nodes_padT": nodes_padT, "nodes_locT": nodes_locT,
            "ws_mat": ws_mat, "wr_mat": wr_mat,
            "wsb_rep": wsb_rep, "wrb_rep": wrb_rep,
            "attn_rep": attn_rep,
            "sidx": sidx, "ohh": ohh, "ohht": ohht,
        })

    trace = bool(int(os.environ.get("GAT_TRACE", "0")))
    res = bass_utils.run_bass_kernel_spmd(nc, in_maps,
                                          core_ids=list(range(NCORE)),
                                          trace=trace)
    if trace:
        kernel.last_profile = res
    out = np.empty((N_NODES, F), np.float32)
    for c in range(NCORE):
        out[c * NPC:(c + 1) * NPC] = np.asarray(res.results[c]["out_d"])[:NPC]
    return out
